# revision 1
# baseline (speedup 1.0000x reference)
"""Trainium2 Bass kernel for GemmaAttention (B=2, S=2048, HID=1024, NH=4, HD=256).

Sharding: 8 cores = batch(2) x heads(4). Each core computes one (b, h):
  q/k/v projections for its head, RoPE, causal attention, and a partial
  output projection [S, HID]; the host sums the 4 per-head partials per batch.

Device-side layout choices (host-side prep is free):
  - hidden passed transposed: xT [HID, S] so the contraction dim (HID) lies on
    partitions for the QKV projections.
  - Wq/Wk rows are permuted to "rotate-half" RoPE layout (evens then odds) so
    RoPE acts on partition-halves of qT/kT [HD, S]; softmax scale folded into Wq.
  - Scores are computed transposed, ST[j, i] = (q_i . k_j), so that:
      * exp needs no per-row bias (no max subtraction; scores are O(5) here)
      * the softmax denominator l[i] = sum_j P[j,i] is a ones-vector matmul
      * P.T is exactly what the PV matmul needs as rhs -> no transposes at all
  - Causal structure: only lower-triangle (j<=i) tiles are computed; diagonal
    tiles get a precomputed binary mask after exp. (If the provided mask is
    not the standard causal -1e9 mask, a generic fallback loops over all
    tiles and adds the provided mask before exp.)
  - All matmul operands are bitcast to float32r: fp32 data read at full PE
    rate (FP22 multiply, fp32 accumulate) instead of the 4x-slower true-fp32
    4-pass mode.
"""

import sys

sys.path.insert(0, "/opt/trn_rl_repo")

import numpy as np

import concourse.bacc as bacc
import concourse.bass as bass
import concourse.mybir as mybir
import concourse.tile as tile
from concourse.bass_utils import run_bass_kernel_spmd


def _ensure_ntff_hook():
    """This image's ``antenv`` lacks ``axon_hooks`` (bass_utils imports it for
    trace=True). Inject an equivalent module driving NTFF profiling via the
    libaxon C ABI (mirrors trn_agent_boot._ntff_profile_via_ctypes)."""
    import types, ctypes, contextlib, os

    if "antenv.axon_hooks" in sys.modules:
        return
    so_path = "/opt/axon/libaxon_pjrt.so"
    hook = None
    if os.path.exists(so_path):
        lib = ctypes.CDLL(so_path)
        if hasattr(lib, "axon_start_nrt_profile"):
            lib.axon_start_nrt_profile.argtypes = [
                ctypes.POINTER(ctypes.c_int64),
                ctypes.c_size_t,
            ]
            lib.axon_start_nrt_profile.restype = ctypes.c_int64
            lib.axon_stop_nrt_profile.argtypes = [ctypes.c_char_p]
            lib.axon_stop_nrt_profile.restype = ctypes.c_int64

            @contextlib.contextmanager
            def _hook(output_dir, device_ids):
                import jax

                jax.devices()
                if device_ids:
                    ids = (ctypes.c_int64 * len(device_ids))(*device_ids)
                    rc = lib.axon_start_nrt_profile(ids, len(device_ids))
                else:
                    rc = lib.axon_start_nrt_profile(None, 0)
                if rc != 0:
                    raise RuntimeError(f"axon_start_nrt_profile rc={rc}")
                try:
                    yield
                finally:
                    n = lib.axon_stop_nrt_profile(str(output_dir).encode())
                    if n < 0:
                        raise RuntimeError(f"axon_stop_nrt_profile rc={n}")
                    print(f"profile: {n} file(s) written to {output_dir}")

            hook = _hook

    mod = types.ModuleType("antenv.axon_hooks")
    _state = {"hook": hook}
    mod.set_axon_ntff_profile_hook = lambda h: _state.__setitem__("hook", h)
    mod.get_axon_ntff_profile_hook = lambda: _state["hook"]
    sys.modules["antenv.axon_hooks"] = mod
    import antenv

    antenv.axon_hooks = mod


B, S, HID = 2, 2048, 1024
NH, HD = 4, 256
SCALE = HD**-0.5
P = 128
CH = 512  # i-chunk width (and matmul free-dim)

_cache = {}
F32R = mybir.dt.float32r




def build_nc(s=S, causal=True, **bacc_kwargs):
    """Emit the single-core program (SPMD: all 8 cores run this)."""
    nsc = s // CH          # number of i-chunks
    njt = s // P           # number of j-tiles
    kt_n = HID // P        # contraction tiles for projections
    ntd = CH // P          # i-subtiles per chunk / diagonal j-tiles per chunk

    nc = bacc.Bacc(**bacc_kwargs)
    f32 = mybir.dt.float32
    xT = nc.declare_dram_parameter("xT", [HID, s], F32R, isOutput=False)
    wq = nc.declare_dram_parameter("wq", [HID, HD], F32R, isOutput=False)
    wk = nc.declare_dram_parameter("wk", [HID, HD], F32R, isOutput=False)
    wv = nc.declare_dram_parameter("wv", [HID, HD], F32R, isOutput=False)
    wo = nc.declare_dram_parameter("wo", [HD, HID], F32R, isOutput=False)
    ones = nc.declare_dram_parameter("ones", [P, 2], F32R, isOutput=False)
    frT = nc.declare_dram_parameter("frT", [P, s], f32, isOutput=False)
    fiT = nc.declare_dram_parameter("fiT", [P, s], f32, isOutput=False)
    if causal:
        mk = nc.declare_dram_parameter("mk", [P, ntd, CH], f32, isOutput=False)
    else:
        mk = nc.declare_dram_parameter("mk", [s, s], f32, isOutput=False)
    out = nc.declare_dram_parameter("out", [s, HID], f32, isOutput=True)

    with tile.TileContext(nc) as tc:
        with (
            tc.tile_pool(name="consts", bufs=1) as consts,
            tc.tile_pool(name="qkv", bufs=1) as qkv,
        ):
            # ---- constant + input loads (order matters: q/k weights and xT
            # first so projection matmuls start as soon as tiles land) ----
            wq_sb = consts.tile([P, kt_n, HD], F32R)
            wk_sb = consts.tile([P, kt_n, HD], F32R)
            nc.sync.dma_start(out=wq_sb, in_=wq.rearrange("(o p) f -> p o f", p=P))
            nc.sync.dma_start(out=wk_sb, in_=wk.rearrange("(o p) f -> p o f", p=P))

            xp = tc.tile_pool(name="xp", bufs=1)
            xT_sb = xp.__enter__().tile([P, kt_n, s], F32R)
            xpool = xp  # closed manually after phase 1
            for kt in range(kt_n):
                nc.sync.dma_start(
                    out=xT_sb[:, kt, :], in_=xT[kt * P : (kt + 1) * P, :]
                )

            frT_sb = consts.tile([P, s], f32)
            fiT_sb = consts.tile([P, s], f32)
            nc.sync.dma_start(out=frT_sb, in_=frT[:])
            nc.sync.dma_start(out=fiT_sb, in_=fiT[:])
            wv_sb = consts.tile([P, kt_n, HD], F32R)
            wo_sb = consts.tile([P, HD // P, HID], F32R)
            nc.sync.dma_start(out=wv_sb, in_=wv.rearrange("(o p) f -> p o f", p=P))
            nc.sync.dma_start(out=wo_sb, in_=wo.rearrange("(o p) f -> p o f", p=P))
            if causal:
                mk_sb = consts.tile([P, ntd, CH], f32)
                nc.sync.dma_start(out=mk_sb, in_=mk[:])
            ones_sb = consts.tile([P, 2], F32R)
            nc.sync.dma_start(out=ones_sb, in_=ones[:])

            # persistent activations
            qrT_sb = qkv.tile([P, HD // P, s], F32R)  # rope'd qT (d on partitions)
            krT_sb = qkv.tile([P, HD // P, s], F32R)
            v_sb = qkv.tile([P, njt, HD], F32R)       # v[j, e] per j-tile

            # ================= phase 1: projections + rope =================
            with (
                tc.tile_pool(name="ps_q", bufs=2, space="PSUM") as ps_q,
                tc.tile_pool(name="ps_v", bufs=2, space="PSUM") as ps_v,
                tc.tile_pool(name="rtmp", bufs=3) as rtmp,
            ):
                # q and k projections, chunk by chunk, rope fused from psum
                for wsb, dst in ((wq_sb, qrT_sb), (wk_sb, krT_sb)):
                    for c in range(nsc):
                        cs = slice(c * CH, (c + 1) * CH)
                        ps0 = ps_q.tile([P, CH], f32, tag="pj0")
                        ps1 = ps_q.tile([P, CH], f32, tag="pj1")
                        for m, ps in ((0, ps0), (1, ps1)):
                            for kt in range(kt_n):
                                nc.tensor.matmul(
                                    ps,
                                    wsb[:, kt, m * P : (m + 1) * P],
                                    xT_sb[:, kt, cs],
                                    start=(kt == 0),
                                    stop=(kt == kt_n - 1),
                                )
                        fr = frT_sb[:, cs]
                        fi = fiT_sb[:, cs]
                        t0 = rtmp.tile([P, CH], f32, tag="t0")
                        t1 = rtmp.tile([P, CH], f32, tag="t1")
                        # dst0 = ps0*fr - ps1*fi ; dst1 = ps0*fi + ps1*fr
                        nc.vector.tensor_mul(dst[:, 0, cs], ps0, fr)
                        nc.vector.tensor_mul(t0, ps1, fi)
                        nc.vector.tensor_sub(dst[:, 0, cs], dst[:, 0, cs], t0)
                        nc.vector.tensor_mul(dst[:, 1, cs], ps0, fi)
                        nc.vector.tensor_mul(t1, ps1, fr)
                        nc.vector.tensor_add(dst[:, 1, cs], dst[:, 1, cs], t1)

                # v projection: v[j, e] tiles
                for st in range(njt):
                    psv = ps_v.tile([P, HD], f32, tag="pv")
                    for kt in range(kt_n):
                        nc.tensor.matmul(
                            psv,
                            xT_sb[:, kt, st * P : (st + 1) * P],
                            wv_sb[:, kt, :],
                            start=(kt == 0),
                            stop=(kt == kt_n - 1),
                        )
                    nc.vector.tensor_copy(v_sb[:, st, :], psv)

            xpool.__exit__(None, None, None)

            # ================= phase 2: attention + out proj =================
            with (
                tc.tile_pool(name="ps_st", bufs=2, space="PSUM") as ps_st,
                tc.tile_pool(name="ps_at", bufs=1, space="PSUM") as ps_at,
                tc.tile_pool(name="ps_l", bufs=1, space="PSUM") as ps_l,
                tc.tile_pool(name="ps_o", bufs=2, space="PSUM") as ps_o,
                tc.tile_pool(name="ps_rl", bufs=1, space="PSUM") as ps_rl,
                tc.tile_pool(name="work", bufs=2) as work,
                tc.tile_pool(name="pwork", bufs=3) as pwork,
                tc.tile_pool(name="ob", bufs=3) as obp,
            ):
                def finalize(c, attn_sb, l_sb):
                    """rl chain + out projection + store for chunk c (issued
                    mid-way through chunk c+1's attention so the serial DVE/PE
                    latency hides behind attention matmuls)."""
                    # fp32r matmul ISA needs even dst/moving free counts:
                    # write each transposed value twice ([P,2] per isub)
                    rl_ps = ps_rl.tile([P, 2 * ntd], f32, tag="rl")
                    for isub in range(ntd):
                        nc.tensor.matmul(
                            rl_ps[:, 2 * isub : 2 * isub + 2],
                            l_sb[:, isub * P : (isub + 1) * P],
                            ones_sb[0:1, 0:2],
                            start=True,
                            stop=True,
                        )
                    rl_sb = work.tile([P, 2 * ntd], f32, tag="rlsb")
                    nc.vector.reciprocal(rl_sb, rl_ps)
                    for isub in range(ntd):
                        ob = obp.tile([P, HID], f32, tag="ob")
                        for fc in range(HID // CH):
                            ops = ps_o.tile([P, CH], f32, tag="o")
                            for et in range(HD // P):
                                nc.tensor.matmul(
                                    ops,
                                    attn_sb[:, et, isub * P : (isub + 1) * P],
                                    wo_sb[:, et, fc * CH : (fc + 1) * CH],
                                    start=(et == 0),
                                    stop=(et == HD // P - 1),
                                )
                            nc.vector.tensor_scalar_mul(
                                ob[:, fc * CH : (fc + 1) * CH],
                                ops,
                                rl_sb[:, 2 * isub : 2 * isub + 1],
                            )
                        nc.sync.dma_start(
                            out=out[c * CH + isub * P : c * CH + (isub + 1) * P, :],
                            in_=ob,
                        )

                pending = None
                for c in range(nsc):
                    ics = slice(c * CH, (c + 1) * CH)
                    attn_ps = ps_at.tile([P, HD // P, CH], f32, tag="at")
                    l_ps = ps_l.tile([1, CH], f32, tag="l")
                    jmax = njt if not causal else ntd * c + ntd
                    for t in range(jmax):
                        stp = ps_st.tile([P, CH], f32, tag="st")
                        for dt in range(HD // P):
                            nc.tensor.matmul(
                                stp,
                                krT_sb[:, dt, t * P : (t + 1) * P],
                                qrT_sb[:, dt, ics],
                                start=(dt == 0),
                                stop=(dt == HD // P - 1),
                            )
                        p_sb = pwork.tile([P, CH], F32R, tag="p")
                        if not causal:
                            # add provided additive mask (transposed view [j, i])
                            mrow = mk[t * P : (t + 1) * P, ics]
                            m_sb = pwork.tile([P, CH], f32, tag="m")
                            nc.sync.dma_start(out=m_sb, in_=mrow)
                            nc.vector.tensor_add(stp, stp, m_sb)
                        nc.scalar.activation(
                            p_sb, stp, mybir.ActivationFunctionType.Exp
                        )
                        if causal and t >= ntd * c:
                            nc.vector.tensor_mul(p_sb, p_sb, mk_sb[:, t - ntd * c, :])
                        first, last = t == 0, t == jmax - 1
                        for et in range(HD // P):
                            nc.tensor.matmul(
                                attn_ps[:, et, :],
                                v_sb[:, t, et * P : (et + 1) * P],
                                p_sb,
                                start=first,
                                stop=last,
                            )
                        nc.tensor.matmul(
                            l_ps, ones_sb[:, 0:1], p_sb, start=first, stop=last
                        )
                        if t == 2 and pending is not None:
                            finalize(*pending)
                            pending = None

                    # drain psums immediately (frees banks for next chunk)
                    attn_sb = work.tile([P, HD // P, CH], F32R, tag="attn")
                    nc.vector.tensor_copy(attn_sb, attn_ps)
                    l_sb = work.tile([1, CH], F32R, tag="lsb")
                    nc.vector.tensor_copy(l_sb, l_ps)
                    if pending is not None:
                        finalize(*pending)
                    pending = (c, attn_sb, l_sb)
                finalize(*pending)

    nc.compile()
    return nc


def _perm():
    return np.concatenate([np.arange(0, HD, 2), np.arange(1, HD, 2)])


def make_core_inputs(hidden_states, freqs_real, freqs_imag, mask, W_qkv, W_o, causal):
    """Host-side shard + relayout. Returns list of 8 in_maps (core = b*NH + h)."""
    perm = _perm()
    frT = np.ascontiguousarray(freqs_real.T.astype(np.float32))
    fiT = np.ascontiguousarray(freqs_imag.T.astype(np.float32))
    if causal:
        r = np.arange(P)[:, None, None]
        o = np.arange(CH // P)[None, :, None]
        cc = np.arange(CH)[None, None, :]
        mk = (cc >= r + P * o).astype(np.float32)
        mk = np.ascontiguousarray(mk)
    else:
        mk = np.ascontiguousarray(mask[0, 0].T.astype(np.float32))  # [j, i]
    in_maps = []
    for b in range(B):
        xT = np.ascontiguousarray(hidden_states[b].T.astype(np.float32))
        for h in range(NH):
            wq_h = W_qkv[h * HD : (h + 1) * HD, :]
            wk_h = W_qkv[HID + h * HD : HID + (h + 1) * HD, :]
            wv_h = W_qkv[2 * HID + h * HD : 2 * HID + (h + 1) * HD, :]
            wo_h = W_o[:, h * HD : (h + 1) * HD]
            in_maps.append(
                {
                    "xT": xT,
                    "wq": np.ascontiguousarray(
                        (wq_h[perm, :] * SCALE).T.astype(np.float32)
                    ),
                    "wk": np.ascontiguousarray(wk_h[perm, :].T.astype(np.float32)),
                    "wv": np.ascontiguousarray(wv_h.T.astype(np.float32)),
                    "wo": np.ascontiguousarray(wo_h.T.astype(np.float32)),
                    "frT": frT,
                    "fiT": fiT,
                    "mk": mk,
                    "ones": np.ones((P, 2), dtype=np.float32),
                }
            )
    return in_maps


def _is_causal(mask):
    m = np.asarray(mask)
    if m.shape != (1, 1, S, S):
        return False
    causal = np.tril(np.ones((S, S), dtype=bool))
    expect = np.where(causal, np.float32(0.0), np.float32(-1e9))
    return bool(np.array_equal(m[0, 0], expect))


def kernel(hidden_states, freqs_real, freqs_imag, mask, W_qkv, W_o, _trace=False):
    hidden_states = np.asarray(hidden_states)
    freqs_real = np.asarray(freqs_real)
    freqs_imag = np.asarray(freqs_imag)
    mask = np.asarray(mask)
    W_qkv = np.asarray(W_qkv)
    W_o = np.asarray(W_o)

    if _trace:
        _ensure_ntff_hook()
    causal = _is_causal(mask)
    key = ("nc", causal)
    if key not in _cache:
        _cache[key] = build_nc(S, causal=causal)
    nc = _cache[key]
    in_maps = make_core_inputs(
        hidden_states, freqs_real, freqs_imag, mask, W_qkv, W_o, causal
    )
    res = run_bass_kernel_spmd(nc, in_maps, list(range(B * NH)), trace=_trace)
    outs = [res.results[i]["out"] for i in range(B * NH)]
    full = np.zeros((B, S, HID), dtype=np.float32)
    for b in range(B):
        for h in range(NH):
            full[b] += outs[b * NH + h]
    if _trace:
        return full, res
    return full



# revision 3
# speedup vs baseline: 1.0596x; 1.0596x over previous
"""Trainium2 Bass kernel for GemmaAttention (B=2, S=2048, HID=1024, NH=4, HD=256).

Sharding: 8 cores = batch(2) x heads(4). Each core computes one (b, h):
  q/k/v projections for its head, RoPE, attention, and a partial output
  projection [S, HID]; the host sums the 4 per-head partials per batch.

Design notes (v2 — fused pipeline):
  - Single fused PE stream: proj(c+1) is interleaved with attn(c) and the
    output projection of chunk c-1, so the tensor engine never idles (idle
    resets the PE p-state from 2.4GHz back to 1.2GHz).
  - All matmul operands are bf16 (same PE rate as fp32r at moving>=256 but
    half the DMA/LDWEIGHTS cost and 2x DVE mode for RoPE); PSUM accumulation
    stays fp32 and the softmax denominator/reciprocal stay fp32.
  - Scores are computed transposed, ST[j, i] = (q_i . k_j): exp needs no max
    subtraction, l[i] = sum_j P[j,i] is a ones-stationary matmul, and P^T is
    exactly the PV moving operand (no transposes anywhere).
  - Mask modes compiled on demand:
      nomask : mask == 0 -> full attention, no mask work at all
      causal : standard tril(-1e9) mask -> lower-triangle tiles only, binary
               multiplicative mask on diagonal tiles after exp
      generic: arbitrary additive mask -> host precomputes exp(mask^T) in
               bf16; streamed in and applied multiplicatively after exp
  - Engine placement: exp / psum drains / (1/l) output scaling on ScalarE,
    RoPE tensor-tensor in bf16 on VectorE, matmuls on PE.
"""

import sys

sys.path.insert(0, "/opt/trn_rl_repo")

from collections import deque

import numpy as np
import ml_dtypes

import concourse.bacc as bacc
import concourse.bass as bass
import concourse.mybir as mybir
import concourse.tile as tile
from concourse.bass_utils import run_bass_kernel_spmd


def _ensure_ntff_hook():
    """This image's ``antenv`` lacks ``axon_hooks`` (bass_utils imports it for
    trace=True). Inject an equivalent module driving NTFF profiling via the
    libaxon C ABI (mirrors trn_agent_boot._ntff_profile_via_ctypes)."""
    import types, ctypes, contextlib, os

    if "antenv.axon_hooks" in sys.modules:
        return
    so_path = "/opt/axon/libaxon_pjrt.so"
    hook = None
    if os.path.exists(so_path):
        lib = ctypes.CDLL(so_path)
        if hasattr(lib, "axon_start_nrt_profile"):
            lib.axon_start_nrt_profile.argtypes = [
                ctypes.POINTER(ctypes.c_int64),
                ctypes.c_size_t,
            ]
            lib.axon_start_nrt_profile.restype = ctypes.c_int64
            lib.axon_stop_nrt_profile.argtypes = [ctypes.c_char_p]
            lib.axon_stop_nrt_profile.restype = ctypes.c_int64

            @contextlib.contextmanager
            def _hook(output_dir, device_ids):
                import jax

                jax.devices()
                if device_ids:
                    ids = (ctypes.c_int64 * len(device_ids))(*device_ids)
                    rc = lib.axon_start_nrt_profile(ids, len(device_ids))
                else:
                    rc = lib.axon_start_nrt_profile(None, 0)
                if rc != 0:
                    raise RuntimeError(f"axon_start_nrt_profile rc={rc}")
                try:
                    yield
                finally:
                    n = lib.axon_stop_nrt_profile(str(output_dir).encode())
                    if n < 0:
                        raise RuntimeError(f"axon_stop_nrt_profile rc={n}")
                    print(f"profile: {n} file(s) written to {output_dir}")

            hook = _hook

    mod = types.ModuleType("antenv.axon_hooks")
    _state = {"hook": hook}
    mod.set_axon_ntff_profile_hook = lambda h: _state.__setitem__("hook", h)
    mod.get_axon_ntff_profile_hook = lambda: _state["hook"]
    sys.modules["antenv.axon_hooks"] = mod
    import antenv

    antenv.axon_hooks = mod


B, S, HID = 2, 2048, 1024
NH, HD = 4, 256
SCALE = HD**-0.5
P = 128
CH = 512          # i-chunk width (and matmul moving free-dim)
NSC = S // CH     # 4 i-chunks
NJT = S // P      # 16 j-tiles
KT = HID // P     # 8 contraction tiles for projections
ND = CH // P      # 4 i-subtiles per chunk

_cache = {}
BF16 = mybir.dt.bfloat16
F32R = mybir.dt.float32r
NPBF16 = ml_dtypes.bfloat16


def build_nc(mode):
    """Emit the single-core program (SPMD: all 8 cores run this).

    mode: "nomask" (full attention), "causal", or "generic" (streamed
    multiplicative exp(mask))."""
    assert mode in ("nomask", "causal", "generic")
    nc = bacc.Bacc()
    f32 = mybir.dt.float32
    Exp = mybir.ActivationFunctionType.Exp

    xT = nc.declare_dram_parameter("xT", [KT, P, S], BF16, isOutput=False)
    wq = nc.declare_dram_parameter("wq", [P, KT, HD], BF16, isOutput=False)
    wk = nc.declare_dram_parameter("wk", [P, KT, HD], BF16, isOutput=False)
    wv = nc.declare_dram_parameter("wv", [P, KT, HD], BF16, isOutput=False)
    wo = nc.declare_dram_parameter("wo", [P, HD // P, HID], BF16, isOutput=False)
    frT = nc.declare_dram_parameter("frT", [P, S], BF16, isOutput=False)
    fiT = nc.declare_dram_parameter("fiT", [P, S], BF16, isOutput=False)
    ones = nc.declare_dram_parameter("ones", [P, 2], BF16, isOutput=False)
    ones2 = nc.declare_dram_parameter("ones2", [1, 2], F32R, isOutput=False)
    if mode == "causal":
        mk = nc.declare_dram_parameter("mk", [P, ND, CH], BF16, isOutput=False)
    elif mode == "generic":
        mke = nc.declare_dram_parameter("mke", [P, NJT, S], BF16, isOutput=False)
    out = nc.declare_dram_parameter("out", [S, HID], f32, isOutput=True)

    with tile.TileContext(nc) as tc:
        with (
            tc.tile_pool(name="consts", bufs=1) as consts,
            tc.tile_pool(name="xp", bufs=1) as xp,
            tc.tile_pool(name="qk", bufs=1) as qk,
            tc.tile_pool(name="rst", bufs=4) as rst,
            tc.tile_pool(name="pw", bufs=3) as pw,
            tc.tile_pool(name="aw", bufs=2) as aw,
            tc.tile_pool(name="lw", bufs=2) as lw,
            tc.tile_pool(name="obp", bufs=3) as obp,
            tc.tile_pool(name="psp", bufs=1, space="PSUM") as psp,
        ):
            # ---------------- constant + input loads ----------------
            # Order matters: wq + xT chunk 0 first so the PE starts ~1.5us in.
            wq_sb = consts.tile([P, KT, HD], BF16)
            nc.sync.dma_start(out=wq_sb, in_=wq[:])
            xcs = []

            def load_xc(c):
                xc = xp.tile([P, KT, CH], BF16, tag=f"xc{c}")
                for kt in range(KT):
                    nc.sync.dma_start(
                        out=xc[:, kt, :], in_=xT[kt, :, c * CH : (c + 1) * CH]
                    )
                xcs.append(xc)

            load_xc(0)
            wk_sb = consts.tile([P, KT, HD], BF16)
            nc.sync.dma_start(out=wk_sb, in_=wk[:])
            frT_sb = consts.tile([P, S], BF16)
            fiT_sb = consts.tile([P, S], BF16)
            nc.sync.dma_start(out=frT_sb, in_=frT[:])
            nc.sync.dma_start(out=fiT_sb, in_=fiT[:])
            wv_sb = consts.tile([P, KT, HD], BF16)
            nc.sync.dma_start(out=wv_sb, in_=wv[:])
            load_xc(1)
            wo_sb = consts.tile([P, HD // P, HID], BF16)
            nc.sync.dma_start(out=wo_sb, in_=wo[:])
            if mode == "causal":
                mk_sb = consts.tile([P, ND, CH], BF16)
                nc.sync.dma_start(out=mk_sb, in_=mk[:])
            ones_sb = consts.tile([P, 2], BF16)
            nc.sync.dma_start(out=ones_sb, in_=ones[:])
            ones2_sb = consts.tile([1, 2], F32R)
            nc.sync.dma_start(out=ones2_sb, in_=ones2[:])
            load_xc(2)
            load_xc(3)

            # persistent activations
            qrT_sb = qk.tile([P, HD // P, S], BF16)   # rope'd qT (d on partitions)
            krT_sb = qk.tile([P, HD // P, S], BF16)
            v_sb = qk.tile([P, NJT, HD], BF16)        # v[j, e] per j-tile

            # ---------------- building blocks ----------------
            def proj(c):
                """q/k projections + fused rope for i-chunk c; v for its 4
                i-tiles. All outputs land in SBUF as bf16."""
                xc = xcs[c]
                cs = slice(c * CH, (c + 1) * CH)
                for wsb, dst in ((wq_sb, qrT_sb), (wk_sb, krT_sb)):
                    ps0 = psp.tile([P, CH], f32, tag="st", bufs=3, name="ps0")
                    ps1 = psp.tile([P, CH], f32, tag="st", bufs=3, name="ps1")
                    for m, ps in ((0, ps0), (1, ps1)):
                        for kt in range(KT):
                            nc.tensor.matmul(
                                ps,
                                wsb[:, kt, m * P : (m + 1) * P],
                                xc[:, kt, :],
                                start=(kt == 0),
                                stop=(kt == KT - 1),
                            )
                    # stage psum -> sbuf bf16 on ScalarE, then rope on DVE in
                    # bf16 (2x mode): dst0 = s0*fr - s1*fi; dst1 = s0*fi + s1*fr
                    s0 = rst.tile([P, CH], BF16, tag="rs", bufs=4, name="s0")
                    s1 = rst.tile([P, CH], BF16, tag="rs", bufs=4, name="s1")
                    nc.scalar.copy(s0, ps0)
                    nc.scalar.copy(s1, ps1)
                    fr = frT_sb[:, cs]
                    fi = fiT_sb[:, cs]
                    t0 = rst.tile([P, CH], BF16, tag="rt", bufs=2, name="t0")
                    t1 = rst.tile([P, CH], BF16, tag="rt", bufs=2, name="t1")
                    nc.vector.tensor_mul(dst[:, 0, cs], s0, fr)
                    nc.vector.tensor_mul(t0, s1, fi)
                    nc.vector.tensor_sub(dst[:, 0, cs], dst[:, 0, cs], t0)
                    nc.vector.tensor_mul(dst[:, 1, cs], s0, fi)
                    nc.vector.tensor_mul(t1, s1, fr)
                    nc.vector.tensor_add(dst[:, 1, cs], dst[:, 1, cs], t1)
                for sl in range(ND):
                    st = ND * c + sl
                    psv = psp.tile([P, HD], f32, tag="o", bufs=2, name="psv")
                    for kt in range(KT):
                        nc.tensor.matmul(
                            psv,
                            xc[:, kt, sl * P : (sl + 1) * P],
                            wv_sb[:, kt, :],
                            start=(kt == 0),
                            stop=(kt == KT - 1),
                        )
                    nc.scalar.copy(v_sb[:, st, :], psv)

            def fin_setup(l_sb):
                """Transpose l to partitions (tiny matmuls) + reciprocal."""
                rl_ps = psp.tile([P, 2 * ND], f32, tag="o", bufs=2, name="rlps")
                for i in range(ND):
                    nc.tensor.matmul(
                        rl_ps[:, 2 * i : 2 * i + 2],
                        l_sb[:, i * P : (i + 1) * P],
                        ones2_sb,
                        start=True,
                        stop=True,
                    )
                rl_sb = lw.tile([P, 2 * ND], f32, tag="rl", bufs=2, name="rlsb")
                nc.vector.reciprocal(rl_sb, rl_ps)
                return rl_sb

            def fin_isub(c, attn_sb, rl_sb, isub):
                """Output projection + 1/l scaling + store for one i-subtile."""
                ob = obp.tile([P, HID], f32, tag="ob", bufs=3, name="ob")
                for fc in range(HID // CH):
                    ops = psp.tile([P, CH], f32, tag="o", bufs=2, name="ops")
                    for et in range(HD // P):
                        nc.tensor.matmul(
                            ops,
                            attn_sb[:, et, isub * P : (isub + 1) * P],
                            wo_sb[:, et, fc * CH : (fc + 1) * CH],
                            start=(et == 0),
                            stop=(et == HD // P - 1),
                        )
                    nc.scalar.mul(
                        ob[:, fc * CH : (fc + 1) * CH],
                        ops,
                        rl_sb[:, 2 * isub : 2 * isub + 1],
                    )
                nc.sync.dma_start(
                    out=out[c * CH + isub * P : c * CH + (isub + 1) * P, :],
                    in_=ob,
                )

            def attn(r, pending):
                """Attention for i-chunk r (transposed-scores flash-less
                softmax). pending = (c, attn_sb, rl_sb) of chunk c=r-1 whose
                output projection is interleaved at t=1..4."""
                jmax = ND * r + ND if mode == "causal" else NJT
                ics = slice(r * CH, (r + 1) * CH)
                attn_ps = psp.tile([P, HD // P, CH], f32, tag="big", bufs=1, name="at")
                l_ps = psp.tile([1, CH], f32, tag="l", bufs=1, name="l")

                def mk_p(t):
                    stp = psp.tile([P, CH], f32, tag="st", bufs=3, name="stp")
                    for dt in range(HD // P):
                        nc.tensor.matmul(
                            stp,
                            krT_sb[:, dt, t * P : (t + 1) * P],
                            qrT_sb[:, dt, ics],
                            start=(dt == 0),
                            stop=(dt == HD // P - 1),
                        )
                    p = pw.tile([P, CH], BF16, tag="p", bufs=3, name="p")
                    nc.scalar.activation(p, stp, Exp)
                    if mode == "causal" and t >= ND * r:
                        nc.vector.tensor_mul(p, p, mk_sb[:, t - ND * r, :])
                    elif mode == "generic":
                        me = pw.tile([P, CH], BF16, tag="me", bufs=3, name="me")
                        nc.sync.dma_start(out=me, in_=mke[:, t, ics])
                        nc.vector.tensor_mul(p, p, me)
                    return p

                pq = deque()
                pq.append(mk_p(0))
                pq.append(mk_p(1))
                for t in range(jmax):
                    if t + 2 < jmax:
                        pq.append(mk_p(t + 2))
                    p = pq.popleft()
                    first, last = t == 0, t == jmax - 1
                    for et in range(HD // P):
                        nc.tensor.matmul(
                            attn_ps[:, et, :],
                            v_sb[:, t, et * P : (et + 1) * P],
                            p,
                            start=first,
                            stop=last,
                        )
                    nc.tensor.matmul(
                        l_ps, ones_sb[:, 0:1], p, start=first, stop=last
                    )
                    if pending is not None and 1 <= t <= ND:
                        fin_isub(pending[0], pending[1], pending[2], t - 1)
                # drain psums (frees banks for the next chunk)
                attn_sb = aw.tile([P, HD // P, CH], BF16, tag="at", bufs=2, name="atsb")
                nc.scalar.copy(attn_sb, attn_ps)
                l_sb = lw.tile([1, CH], F32R, tag="l", bufs=2, name="lsb")
                nc.vector.tensor_copy(l_sb, l_ps)
                return (r, attn_sb, l_sb)

            # ---------------- fused main pipeline ----------------
            # Causal: attn(r) only needs k/v chunks <= r, so proj(r+1) can be
            # interleaved after attn(r-1). Full attention (nomask/generic):
            # attn(0) reads ALL k/v chunks, so every projection must be
            # emitted first (still one continuous PE stream).
            proj(0)
            if mode != "causal":
                for c in range(1, NSC):
                    proj(c)
            drained = None
            for r in range(NSC):
                if mode == "causal" and r + 1 < NSC:
                    proj(r + 1)
                pend = None
                if drained is not None:
                    rl_sb = fin_setup(drained[2])
                    pend = (drained[0], drained[1], rl_sb)
                drained = attn(r, pend)
            rl_sb = fin_setup(drained[2])
            for isub in range(ND):
                fin_isub(drained[0], drained[1], rl_sb, isub)

    nc.compile()
    return nc


def _perm():
    return np.concatenate([np.arange(0, HD, 2), np.arange(1, HD, 2)])


def make_core_inputs(hidden_states, freqs_real, freqs_imag, mask, W_qkv, W_o, mode):
    """Host-side shard + relayout (free). Returns 8 in_maps (core = b*NH + h)."""
    perm = _perm()
    frT = np.ascontiguousarray(freqs_real.T).astype(NPBF16)
    fiT = np.ascontiguousarray(freqs_imag.T).astype(NPBF16)
    extras = {}
    if mode == "causal":
        r = np.arange(P)[:, None, None]
        o = np.arange(ND)[None, :, None]
        cc = np.arange(CH)[None, None, :]
        extras["mk"] = np.ascontiguousarray((cc >= r + P * o)).astype(NPBF16)
    elif mode == "generic":
        # exp(mask^T)[j, i] reshaped to [p, jt, i]
        m = np.exp(np.asarray(mask[0, 0], dtype=np.float64).T)  # [j, i]
        m = m.reshape(NJT, P, S).transpose(1, 0, 2)
        extras["mke"] = np.ascontiguousarray(m).astype(NPBF16)
    in_maps = []
    for b in range(B):
        xTr = np.ascontiguousarray(
            hidden_states[b].T.reshape(KT, P, S)
        ).astype(NPBF16)
        for h in range(NH):
            wq_h = (W_qkv[h * HD : (h + 1) * HD, :][perm, :] * SCALE).T
            wk_h = W_qkv[HID + h * HD : HID + (h + 1) * HD, :][perm, :].T
            wv_h = W_qkv[2 * HID + h * HD : 2 * HID + (h + 1) * HD, :].T
            wo_h = W_o[:, h * HD : (h + 1) * HD].T
            in_maps.append(
                {
                    "xT": xTr,
                    "wq": np.ascontiguousarray(
                        wq_h.reshape(KT, P, HD).transpose(1, 0, 2)
                    ).astype(NPBF16),
                    "wk": np.ascontiguousarray(
                        wk_h.reshape(KT, P, HD).transpose(1, 0, 2)
                    ).astype(NPBF16),
                    "wv": np.ascontiguousarray(
                        wv_h.reshape(KT, P, HD).transpose(1, 0, 2)
                    ).astype(NPBF16),
                    "wo": np.ascontiguousarray(
                        wo_h.reshape(HD // P, P, HID).transpose(1, 0, 2)
                    ).astype(NPBF16),
                    "frT": frT,
                    "fiT": fiT,
                    "ones": np.ones((P, 2), dtype=NPBF16),
                    "ones2": np.ones((1, 2), dtype=np.float32),
                    **extras,
                }
            )
    return in_maps


def _mask_mode(mask):
    m = np.asarray(mask)
    if m.shape != (1, 1, S, S):
        return "generic"
    if not np.any(m):
        return "nomask"
    causal = np.tril(np.ones((S, S), dtype=bool))
    expect = np.where(causal, np.float32(0.0), np.float32(-1e9))
    if np.array_equal(m[0, 0], expect):
        return "causal"
    return "generic"


def kernel(hidden_states, freqs_real, freqs_imag, mask, W_qkv, W_o, _trace=False):
    hidden_states = np.asarray(hidden_states, dtype=np.float32)
    freqs_real = np.asarray(freqs_real, dtype=np.float32)
    freqs_imag = np.asarray(freqs_imag, dtype=np.float32)
    mask = np.asarray(mask)
    W_qkv = np.asarray(W_qkv, dtype=np.float32)
    W_o = np.asarray(W_o, dtype=np.float32)

    if _trace:
        _ensure_ntff_hook()
    mode = _mask_mode(mask)
    if mode not in _cache:
        _cache[mode] = build_nc(mode)
    nc = _cache[mode]
    in_maps = make_core_inputs(
        hidden_states, freqs_real, freqs_imag, mask, W_qkv, W_o, mode
    )
    res = run_bass_kernel_spmd(nc, in_maps, list(range(B * NH)), trace=_trace)
    outs = [res.results[i]["out"] for i in range(B * NH)]
    full = np.zeros((B, S, HID), dtype=np.float32)
    for b in range(B):
        for h in range(NH):
            full[b] += outs[b * NH + h]
    if _trace:
        return full, res
    return full
